# revision 2
# baseline (speedup 1.0000x reference)
"""MinGRU Trainium2 kernel (nn_MinGRUTriton_77309411812).

Reference computation (B=4, L=8192, D=1024, fp32):
    gates      = sigmoid(x @ Wg.T + bg)
    candidates = tanh   (x @ Wc.T + bc)
    h_t = gates_t * h_{t-1} + candidates_t        (h_0 = 0, scan along L)

Sharding (8 cores, no cross-core communication):
    core c -> batch b = c // 2, output-channel half eh = c % 2 (512 channels).

Host-side shard prep feeds each core transposed operands so the device
kernel needs no transposes at all:
    xT  = x[b].T               [1024(k), 8192(t)]  (k on partitions = matmul
                                                    contraction dim)
    wgT/wcT = W[eh].T          [1024(k), 512(e)]
The matmul output lands as [e(partitions), t(free)] which is exactly the
layout tensor_tensor_scan needs (scan runs along the free dim), and the
[e, t] result is returned as hT per core, un-transposed on the host.

Per 512-wide t-chunk: one 2 MB DMA load of the xT slice, 64 accumulating
fp32r matmuls (PE), sigmoid/tanh straight out of PSUM with fused per-
partition bias (ACT), one tensor_tensor_scan per 128-channel group (DVE,
chained across chunks via initial=prev[:, -1:]), one 1 MB DMA store.
"""

import sys

import numpy as np

try:
    import concourse.bass as bass  # noqa: F401
except ImportError:  # pragma: no cover - path fallback for fresh environments
    sys.path.insert(0, "/opt/trn_rl_repo")

import concourse.bass as bass
import concourse.mybir as mybir
import concourse.tile as tile
from concourse import bacc
from concourse.bass_utils import run_bass_kernel_spmd

B, L, D = 4, 8192, 1024
E = D // 2          # output channels per core
N_CORES = 8
TC = 512            # t-chunk (= matmul moving free dim = PSUM bank)
NK = D // 128       # contraction k-groups
NE = E // 128       # output-channel groups per core
NCH = L // TC       # t-chunks

F32 = mybir.dt.float32
F32R = mybir.dt.float32r

_compiled = None


def _build():
    nc = bacc.Bacc("TRN2", target_bir_lowering=False, debug=False)

    xT = nc.dram_tensor("xT", [D, L], F32R, kind="ExternalInput")
    wgT = nc.dram_tensor("wgT", [D, E], F32R, kind="ExternalInput")
    wcT = nc.dram_tensor("wcT", [D, E], F32R, kind="ExternalInput")
    bgv = nc.dram_tensor("bgv", [E], F32, kind="ExternalInput")
    bcv = nc.dram_tensor("bcv", [E], F32, kind="ExternalInput")
    hT = nc.dram_tensor("hT", [E, L], F32, kind="ExternalOutput")

    with tile.TileContext(nc) as tc, \
            tc.tile_pool(name="wpool", bufs=1) as wpool, \
            tc.tile_pool(name="xpool", bufs=3) as xpool, \
            tc.tile_pool(name="gcpool", bufs=2) as gcpool, \
            tc.tile_pool(name="hpool", bufs=2) as hpool, \
            tc.tile_pool(name="pspool", bufs=4, space="PSUM") as pspool:

        wg_t = wpool.tile([128, NK, E], F32R)
        nc.sync.dma_start(out=wg_t[:], in_=wgT.rearrange("(g p) e -> p g e", p=128))
        wc_t = wpool.tile([128, NK, E], F32R)
        nc.sync.dma_start(out=wc_t[:], in_=wcT.rearrange("(g p) e -> p g e", p=128))
        bg_t = wpool.tile([128, NE], F32)
        nc.sync.dma_start(out=bg_t[:], in_=bgv.rearrange("(g p) -> p g", p=128))
        bc_t = wpool.tile([128, NE], F32)
        nc.sync.dma_start(out=bc_t[:], in_=bcv.rearrange("(g p) -> p g", p=128))

        h_prev = None
        for ci in range(NCH):
            t0 = ci * TC
            x_t = xpool.tile([128, NK, TC], F32R, tag="x")
            nc.sync.dma_start(
                out=x_t[:],
                in_=xT[:, t0:t0 + TC].rearrange("(g p) t -> p g t", p=128),
            )

            g_t = gcpool.tile([128, NE, TC], F32, tag="g")
            c_t = gcpool.tile([128, NE, TC], F32, tag="c")
            for w_t, b_t, out_t, func in (
                (wg_t, bg_t, g_t, mybir.ActivationFunctionType.Sigmoid),
                (wc_t, bc_t, c_t, mybir.ActivationFunctionType.Tanh),
            ):
                for eg in range(NE):
                    ps = pspool.tile([128, TC], F32, tag="ps")
                    for kg in range(NK):
                        nc.tensor.matmul(
                            ps[:],
                            w_t[:, kg, eg * 128:(eg + 1) * 128],
                            x_t[:, kg, :],
                            start=(kg == 0),
                            stop=(kg == NK - 1),
                        )
                    nc.scalar.activation(
                        out_t[:, eg, :], ps[:], func, bias=b_t[:, eg:eg + 1]
                    )

            h_t = hpool.tile([128, NE, TC], F32, tag="h")
            for eg in range(NE):
                init = 0.0 if ci == 0 else h_prev[:, eg, TC - 1:TC]
                nc.vector.tensor_tensor_scan(
                    h_t[:, eg, :],
                    g_t[:, eg, :],
                    c_t[:, eg, :],
                    initial=init,
                    op0=mybir.AluOpType.mult,
                    op1=mybir.AluOpType.add,
                )
            nc.sync.dma_start(
                out=hT[:, t0:t0 + TC].rearrange("(g p) t -> p g t", p=128),
                in_=h_t[:],
            )
            h_prev = h_t

    nc.compile()
    return nc


def _get_compiled():
    global _compiled
    if _compiled is None:
        _compiled = _build()
    return _compiled


def make_in_maps(x, Wg, bg, Wc, bc):
    x = np.ascontiguousarray(np.asarray(x, dtype=np.float32))
    xT = [np.ascontiguousarray(x[b].T) for b in range(B)]
    in_maps = []
    for c in range(N_CORES):
        b, eh = divmod(c, 2)
        sl = slice(eh * E, (eh + 1) * E)
        in_maps.append({
            "xT": xT[b],
            "wgT": np.ascontiguousarray(np.asarray(Wg, np.float32)[sl].T),
            "wcT": np.ascontiguousarray(np.asarray(Wc, np.float32)[sl].T),
            "bgv": np.ascontiguousarray(np.asarray(bg, np.float32)[sl]),
            "bcv": np.ascontiguousarray(np.asarray(bc, np.float32)[sl]),
        })
    return in_maps


def assemble_output(results):
    out = np.empty((B, L, D), np.float32)
    for c in range(N_CORES):
        b, eh = divmod(c, 2)
        out[b, :, eh * E:(eh + 1) * E] = results[c]["hT"].T
    return out


def kernel(x, Wg, bg, Wc, bc, _trace=False, _trace_kwargs=None):
    nc = _get_compiled()
    in_maps = make_in_maps(x, Wg, bg, Wc, bc)
    res = run_bass_kernel_spmd(
        nc, in_maps, list(range(N_CORES)), trace=_trace,
        **(_trace_kwargs or {}),
    )
    out = assemble_output(res.results)
    if _trace:
        kernel.last_results = res
    return out


# revision 3
# speedup vs baseline: 1.1156x; 1.1156x over previous
"""MinGRU Trainium2 kernel (nn_MinGRUTriton_77309411812).

Reference computation (B=4, L=8192, D=1024, fp32):
    gates      = sigmoid(x @ Wg.T + bg)
    candidates = tanh   (x @ Wc.T + bc)
    h_t = gates_t * h_{t-1} + candidates_t        (h_0 = 0, scan along L)

Sharding (8 cores, no cross-core communication):
    core c -> batch b = c // 2, output-channel half eh = c % 2 (512 channels).

Host-side shard prep feeds each core transposed fp16 operands so the
device kernel needs no transposes or casts at all:
    xT  = x[b].T.astype(fp16)  [1024(k), 8192(t)]  (k on partitions = matmul
                                                    contraction dim)
    wgT/wcT = W[eh].T fp16     [1024(k), 512(e)]
fp16 operands run the PE at 1 cycle/row (4x faster than fp32) with a
fully-hidden 2-byte LDWEIGHTS, halve x HBM traffic, and keep absmax
relative error ~5e-4 (e5m10 rounding, fp32 PSUM accumulation; no range
issues: |x| < 6, |W| < 0.2).

The matmul output lands as [e(partitions), t(free)] which is exactly the
layout tensor_tensor_scan needs (scan runs along the free dim), and the
[e, t] result is returned as hT per core, un-transposed on the host.

Per 512-wide t-chunk: one 1 MB DMA load of the xT slice (sync queue), 64
accumulating fp16 matmuls (PE), sigmoid/tanh straight out of PSUM with
fused per-partition bias (ACT), one tensor_tensor_scan per 128-channel
group (DVE, chained across chunks via initial=prev[:, -1:]), one 1 MB
DMA store.  Weight DMAs ride the scalar-engine HWDGE queue and bias
loads the gpsimd queue so the startup loads all overlap.
"""

import sys

import numpy as np

try:
    import concourse.bass as bass  # noqa: F401
except ImportError:  # pragma: no cover - path fallback for fresh environments
    sys.path.insert(0, "/opt/trn_rl_repo")

import concourse.bass as bass
import concourse.mybir as mybir
import concourse.tile as tile
from concourse import bacc
from concourse.bass_utils import run_bass_kernel_spmd

B, L, D = 4, 8192, 1024
E = D // 2          # output channels per core
N_CORES = 8
TC = 512            # t-chunk (= matmul moving free dim = PSUM bank)
NK = D // 128       # contraction k-groups
NE = E // 128       # output-channel groups per core
NCH = L // TC       # t-chunks

F32 = mybir.dt.float32
F16 = mybir.dt.float16

_compiled = None


def _build():
    nc = bacc.Bacc("TRN2", target_bir_lowering=False, debug=False)

    xT = nc.dram_tensor("xT", [D, L], F16, kind="ExternalInput")
    wgT = nc.dram_tensor("wgT", [D, E], F16, kind="ExternalInput")
    wcT = nc.dram_tensor("wcT", [D, E], F16, kind="ExternalInput")
    bgv = nc.dram_tensor("bgv", [E], F32, kind="ExternalInput")
    bcv = nc.dram_tensor("bcv", [E], F32, kind="ExternalInput")
    hT = nc.dram_tensor("hT", [E, L], F32, kind="ExternalOutput")

    with tile.TileContext(nc) as tc, \
            tc.tile_pool(name="wpool", bufs=1) as wpool, \
            tc.tile_pool(name="xpool", bufs=3) as xpool, \
            tc.tile_pool(name="gcpool", bufs=2) as gcpool, \
            tc.tile_pool(name="hpool", bufs=2) as hpool, \
            tc.tile_pool(name="pspool", bufs=4, space="PSUM") as pspool:

        # Weights on the scalar-engine HWDGE queue, biases on gpsimd, so
        # they overlap the first x-chunk load on the sync queue.
        wg_t = wpool.tile([128, NK, E], F16)
        nc.scalar.dma_start(out=wg_t[:], in_=wgT.rearrange("(g p) e -> p g e", p=128))
        wc_t = wpool.tile([128, NK, E], F16)
        nc.scalar.dma_start(out=wc_t[:], in_=wcT.rearrange("(g p) e -> p g e", p=128))
        bg_t = wpool.tile([128, NE], F32)
        nc.gpsimd.dma_start(out=bg_t[:], in_=bgv.rearrange("(g p) -> p g", p=128))
        bc_t = wpool.tile([128, NE], F32)
        nc.gpsimd.dma_start(out=bc_t[:], in_=bcv.rearrange("(g p) -> p g", p=128))

        h_prev = None
        for ci in range(NCH):
            t0 = ci * TC
            x_t = xpool.tile([128, NK, TC], F16, tag="x")
            nc.sync.dma_start(
                out=x_t[:],
                in_=xT[:, t0:t0 + TC].rearrange("(g p) t -> p g t", p=128),
            )

            g_t = gcpool.tile([128, NE, TC], F32, tag="g")
            c_t = gcpool.tile([128, NE, TC], F32, tag="c")
            for w_t, b_t, out_t, func in (
                (wg_t, bg_t, g_t, mybir.ActivationFunctionType.Sigmoid),
                (wc_t, bc_t, c_t, mybir.ActivationFunctionType.Tanh),
            ):
                for eg in range(NE):
                    ps = pspool.tile([128, TC], F32, tag="ps")
                    for kg in range(NK):
                        nc.tensor.matmul(
                            ps[:],
                            w_t[:, kg, eg * 128:(eg + 1) * 128],
                            x_t[:, kg, :],
                            start=(kg == 0),
                            stop=(kg == NK - 1),
                        )
                    nc.scalar.activation(
                        out_t[:, eg, :], ps[:], func, bias=b_t[:, eg:eg + 1]
                    )

            h_t = hpool.tile([128, NE, TC], F32, tag="h")
            last = ci == NCH - 1
            for eg in range(NE):
                init = 0.0 if ci == 0 else h_prev[:, eg, TC - 1:TC]
                nc.vector.tensor_tensor_scan(
                    h_t[:, eg, :],
                    g_t[:, eg, :],
                    c_t[:, eg, :],
                    initial=init,
                    op0=mybir.AluOpType.mult,
                    op1=mybir.AluOpType.add,
                )
                if last:
                    # Per-group stores so the final store (the kernel-tail
                    # gate) only waits on the last scan and is small.
                    nc.sync.dma_start(
                        out=hT[eg * 128:(eg + 1) * 128, t0:t0 + TC],
                        in_=h_t[:, eg, :],
                    )
            if not last:
                nc.sync.dma_start(
                    out=hT[:, t0:t0 + TC].rearrange("(g p) t -> p g t", p=128),
                    in_=h_t[:],
                )
            h_prev = h_t

    nc.compile()
    return nc


def _get_compiled():
    global _compiled
    if _compiled is None:
        _compiled = _build()
    return _compiled


def make_in_maps(x, Wg, bg, Wc, bc):
    x = np.asarray(x, dtype=np.float32)
    xT = [np.ascontiguousarray(x[b].T.astype(np.float16)) for b in range(B)]
    in_maps = []
    for c in range(N_CORES):
        b, eh = divmod(c, 2)
        sl = slice(eh * E, (eh + 1) * E)
        in_maps.append({
            "xT": xT[b],
            "wgT": np.ascontiguousarray(
                np.asarray(Wg, np.float32)[sl].T.astype(np.float16)),
            "wcT": np.ascontiguousarray(
                np.asarray(Wc, np.float32)[sl].T.astype(np.float16)),
            "bgv": np.ascontiguousarray(np.asarray(bg, np.float32)[sl]),
            "bcv": np.ascontiguousarray(np.asarray(bc, np.float32)[sl]),
        })
    return in_maps


def assemble_output(results):
    out = np.empty((B, L, D), np.float32)
    for c in range(N_CORES):
        b, eh = divmod(c, 2)
        out[b, :, eh * E:(eh + 1) * E] = results[c]["hT"].T
    return out


def kernel(x, Wg, bg, Wc, bc, _trace=False, _trace_kwargs=None):
    nc = _get_compiled()
    in_maps = make_in_maps(x, Wg, bg, Wc, bc)
    res = run_bass_kernel_spmd(
        nc, in_maps, list(range(N_CORES)), trace=_trace,
        **(_trace_kwargs or {}),
    )
    out = assemble_output(res.results)
    if _trace:
        kernel.last_results = res
    return out


# revision 5
# speedup vs baseline: 1.1193x; 1.0034x over previous
"""MinGRU Trainium2 kernel (nn_MinGRUTriton_77309411812).

Reference computation (B=4, L=8192, D=1024, fp32):
    gates      = sigmoid(x @ Wg.T + bg)
    candidates = tanh   (x @ Wc.T + bc)
    h_t = gates_t * h_{t-1} + candidates_t        (h_0 = 0, scan along L)

Sharding (8 cores, no cross-core communication):
    core c -> batch b = c // 2, output-channel half eh = c % 2 (512 channels).

Host-side shard prep feeds each core transposed fp16 operands so the
device kernel needs no transposes or casts at all:
    xT  = x[b].T.astype(fp16)  [1024(k), 8192(t)]  (k on partitions = matmul
                                                    contraction dim)
    wgT/wcT = W[eh].T fp16     [1024(k), 512(e)]
fp16 operands run the PE at 1 cycle/row (4x faster than fp32) with a
fully-hidden 2-byte LDWEIGHTS, halve x HBM traffic, and keep absmax
relative error ~5e-4 (e5m10 rounding, fp32 PSUM accumulation; no range
issues: |x| < 6, |W| < 0.2).

The matmul output lands as [e(partitions), t(free)] which is exactly the
layout tensor_tensor_scan needs (scan runs along the free dim), and the
[e, t] result is returned as hT per core, un-transposed on the host.

Per 512-wide t-chunk: one 1 MB DMA load of the xT slice (sync queue), 64
accumulating fp16 matmuls (PE), sigmoid/tanh straight out of PSUM with
fused per-partition bias (ACT), one tensor_tensor_scan per 128-channel
group (DVE, chained across chunks via initial=prev[:, -1:]), one 1 MB
DMA store.  Weight DMAs ride the scalar-engine HWDGE queue and bias
loads the gpsimd queue so the startup loads all overlap.
"""

import sys

import numpy as np

try:
    import concourse.bass as bass  # noqa: F401
except ImportError:  # pragma: no cover - path fallback for fresh environments
    sys.path.insert(0, "/opt/trn_rl_repo")

import concourse.bass as bass
import concourse.mybir as mybir
import concourse.tile as tile
from concourse import bacc
from concourse.bass_utils import run_bass_kernel_spmd

B, L, D = 4, 8192, 1024
E = D // 2          # output channels per core
N_CORES = 8
TC = 512            # t-chunk (= matmul moving free dim = PSUM bank)
NK = D // 128       # contraction k-groups
NE = E // 128       # output-channel groups per core
NCH = L // TC       # t-chunks

F32 = mybir.dt.float32
F16 = mybir.dt.float16

_compiled = None


def _build():
    nc = bacc.Bacc("TRN2", target_bir_lowering=False, debug=False)

    xT = nc.dram_tensor("xT", [D, L], F16, kind="ExternalInput")
    wgT = nc.dram_tensor("wgT", [D, E], F16, kind="ExternalInput")
    wcT = nc.dram_tensor("wcT", [D, E], F16, kind="ExternalInput")
    bias = nc.dram_tensor("bias", [128, 2 * NE], F32, kind="ExternalInput")
    hT = nc.dram_tensor("hT", [E, L], F32, kind="ExternalOutput")

    with tile.TileContext(nc) as tc, \
            tc.tile_pool(name="wpool", bufs=1) as wpool, \
            tc.tile_pool(name="xpool", bufs=3) as xpool, \
            tc.tile_pool(name="gcpool", bufs=2) as gcpool, \
            tc.tile_pool(name="hpool", bufs=2) as hpool, \
            tc.tile_pool(name="pspool", bufs=4, space="PSUM") as pspool:

        # Startup ordering: the first matmul gates on wg + x chunk 0 only.
        # wg rides the scalar HWDGE ring while x0 rides the sync ring (each
        # ring is FIFO, the two drain in parallel); wc/bias/x1.. follow so
        # their bytes don't compete with the gating transfers.
        wg_t = wpool.tile([128, NK, E], F16)
        nc.scalar.dma_start(out=wg_t[:], in_=wgT.rearrange("(g p) e -> p g e", p=128))
        wc_t = wpool.tile([128, NK, E], F16)
        nc.scalar.dma_start(out=wc_t[:], in_=wcT.rearrange("(g p) e -> p g e", p=128))
        b_all = wpool.tile([128, 2 * NE], F32)
        nc.scalar.dma_start(out=b_all[:], in_=bias[:])
        bg_t = b_all[:, 0:NE]
        bc_t = b_all[:, NE:2 * NE]

        # Warm the PE's HAM clock gate (~3.4us of activity releases the
        # 1.2->2.4 GHz throttle) with dummy matmuls on a zeroed tile while
        # the startup DMAs are in flight.
        warm = wpool.tile([128, 512], F16)
        nc.vector.memset(warm[:], 0.0)
        warm_ps = pspool.tile([128, 512], F32, tag="warm", bufs=1)
        for _ in range(16):
            nc.tensor.matmul(warm_ps[:], warm[:, 0:128], warm[:, 0:512],
                             start=True, stop=True)

        h_prev = None
        for ci in range(NCH):
            t0 = ci * TC
            x_t = xpool.tile([128, NK, TC], F16, tag="x")
            nc.sync.dma_start(
                out=x_t[:],
                in_=xT[:, t0:t0 + TC].rearrange("(g p) t -> p g t", p=128),
            )

            g_t = gcpool.tile([128, NE, TC], F32, tag="g")
            c_t = gcpool.tile([128, NE, TC], F32, tag="c")
            for w_t, b_t, out_t, func in (
                (wg_t, bg_t, g_t, mybir.ActivationFunctionType.Sigmoid),
                (wc_t, bc_t, c_t, mybir.ActivationFunctionType.Tanh),
            ):
                for eg in range(NE):
                    ps = pspool.tile([128, TC], F32, tag="ps")
                    for kg in range(NK):
                        nc.tensor.matmul(
                            ps[:],
                            w_t[:, kg, eg * 128:(eg + 1) * 128],
                            x_t[:, kg, :],
                            start=(kg == 0),
                            stop=(kg == NK - 1),
                        )
                    nc.scalar.activation(
                        out_t[:, eg, :], ps[:], func, bias=b_t[:, eg:eg + 1]
                    )

            h_t = hpool.tile([128, NE, TC], F32, tag="h")
            last = ci == NCH - 1
            for eg in range(NE):
                init = 0.0 if ci == 0 else h_prev[:, eg, TC - 1:TC]
                nc.vector.tensor_tensor_scan(
                    h_t[:, eg, :],
                    g_t[:, eg, :],
                    c_t[:, eg, :],
                    initial=init,
                    op0=mybir.AluOpType.mult,
                    op1=mybir.AluOpType.add,
                )
                if last:
                    # Per-group stores so the final store (the kernel-tail
                    # gate) only waits on the last scan and is small.
                    nc.sync.dma_start(
                        out=hT[eg * 128:(eg + 1) * 128, t0:t0 + TC],
                        in_=h_t[:, eg, :],
                    )
            if not last:
                nc.sync.dma_start(
                    out=hT[:, t0:t0 + TC].rearrange("(g p) t -> p g t", p=128),
                    in_=h_t[:],
                )
            h_prev = h_t

    nc.compile()
    return nc


def _get_compiled():
    global _compiled
    if _compiled is None:
        _compiled = _build()
    return _compiled


def make_in_maps(x, Wg, bg, Wc, bc):
    x = np.asarray(x, dtype=np.float32)
    xT = [np.ascontiguousarray(x[b].T.astype(np.float16)) for b in range(B)]
    in_maps = []
    for c in range(N_CORES):
        b, eh = divmod(c, 2)
        sl = slice(eh * E, (eh + 1) * E)
        in_maps.append({
            "xT": xT[b],
            "wgT": np.ascontiguousarray(
                np.asarray(Wg, np.float32)[sl].T.astype(np.float16)),
            "wcT": np.ascontiguousarray(
                np.asarray(Wc, np.float32)[sl].T.astype(np.float16)),
            "bias": np.ascontiguousarray(np.stack(
                [np.asarray(bg, np.float32)[sl].reshape(NE, 128),
                 np.asarray(bc, np.float32)[sl].reshape(NE, 128)],
            ).reshape(2 * NE, 128).T),
        })
    return in_maps


def assemble_output(results):
    out = np.empty((B, L, D), np.float32)
    for c in range(N_CORES):
        b, eh = divmod(c, 2)
        out[b, :, eh * E:(eh + 1) * E] = results[c]["hT"].T
    return out


def kernel(x, Wg, bg, Wc, bc, _trace=False, _trace_kwargs=None):
    nc = _get_compiled()
    in_maps = make_in_maps(x, Wg, bg, Wc, bc)
    res = run_bass_kernel_spmd(
        nc, in_maps, list(range(N_CORES)), trace=_trace,
        **(_trace_kwargs or {}),
    )
    out = assemble_output(res.results)
    if _trace:
        kernel.last_results = res
    return out


# revision 6
# speedup vs baseline: 1.1307x; 1.0101x over previous
"""MinGRU Trainium2 kernel (nn_MinGRUTriton_77309411812).

Reference computation (B=4, L=8192, D=1024, fp32):
    gates      = sigmoid(x @ Wg.T + bg)
    candidates = tanh   (x @ Wc.T + bc)
    h_t = gates_t * h_{t-1} + candidates_t        (h_0 = 0, scan along L)

Sharding (8 cores, no cross-core communication):
    core c -> batch b = c // 2, output-channel half eh = c % 2 (512 channels).

Host-side shard prep feeds each core transposed fp16 operands so the
device kernel needs no transposes or casts at all:
    xT  = x[b].T.astype(fp16)  [1024(k), 8192(t)]  (k on partitions = matmul
                                                    contraction dim)
    wgT/wcT = W[eh].T fp16     [1024(k), 512(e)]
fp16 operands run the PE at 1 cycle/row (4x faster than fp32) with a
fully-hidden 2-byte LDWEIGHTS, halve x HBM traffic, and keep absmax
relative error ~5e-4 (e5m10 rounding, fp32 PSUM accumulation; no range
issues: |x| < 6, |W| < 0.2).

The matmul output lands as [e(partitions), t(free)] which is exactly the
layout tensor_tensor_scan needs (scan runs along the free dim), and the
[e, t] result is returned as hT per core, un-transposed on the host.

Per 512-wide t-chunk: one 1 MB DMA load of the xT slice (sync queue), 64
accumulating fp16 matmuls (PE), sigmoid/tanh straight out of PSUM with
fused per-partition bias (ACT), one tensor_tensor_scan per 128-channel
group (DVE, chained across chunks via initial=prev[:, -1:]), one 1 MB
DMA store.  Weight DMAs ride the scalar-engine HWDGE queue and bias
loads the gpsimd queue so the startup loads all overlap.
"""

import sys

import numpy as np

try:
    import concourse.bass as bass  # noqa: F401
except ImportError:  # pragma: no cover - path fallback for fresh environments
    sys.path.insert(0, "/opt/trn_rl_repo")

import concourse.bass as bass
import concourse.mybir as mybir
import concourse.tile as tile
from concourse import bacc
from concourse.tile import add_dep_helper
from concourse.bass_utils import run_bass_kernel_spmd

B, L, D = 4, 8192, 1024
E = D // 2          # output channels per core
N_CORES = 8
TC = 512            # t-chunk (= matmul moving free dim = PSUM bank)
NK = D // 128       # contraction k-groups
NE = E // 128       # output-channel groups per core
NCH = L // TC       # t-chunks

F32 = mybir.dt.float32
F16 = mybir.dt.float16

_compiled = None


def _build():
    nc = bacc.Bacc("TRN2", target_bir_lowering=False, debug=False)

    xT = nc.dram_tensor("xT", [D, L], F16, kind="ExternalInput")
    wgT = nc.dram_tensor("wgT", [D, E], F16, kind="ExternalInput")
    wcT = nc.dram_tensor("wcT", [D, E], F16, kind="ExternalInput")
    bias = nc.dram_tensor("bias", [128, 2 * NE], F32, kind="ExternalInput")
    hT = nc.dram_tensor("hT", [E, L], F32, kind="ExternalOutput")

    with tile.TileContext(nc) as tc, \
            tc.tile_pool(name="wpool", bufs=1) as wpool, \
            tc.tile_pool(name="xpool", bufs=3) as xpool, \
            tc.tile_pool(name="gcpool", bufs=2) as gcpool, \
            tc.tile_pool(name="hpool", bufs=2) as hpool, \
            tc.tile_pool(name="pspool", bufs=6, space="PSUM") as pspool:

        # Startup ordering: the first matmul gates on wg + x chunk 0 only.
        # wg rides the scalar HWDGE ring while x0 rides the sync ring (each
        # ring is FIFO, the two drain in parallel); wc/bias/x1.. follow so
        # their bytes don't compete with the gating transfers.
        b_all = wpool.tile([128, 2 * NE], F32)
        nc.sync.dma_start(out=b_all[:], in_=bias[:])
        bg_t = b_all[:, 0:NE]
        bc_t = b_all[:, NE:2 * NE]
        wg_t = wpool.tile([128, NK, E], F16)
        i_wg = nc.scalar.dma_start(
            out=wg_t[:], in_=wgT.rearrange("(g p) e -> p g e", p=128))
        wc_t = wpool.tile([128, NK, E], F16)
        i_wc = nc.scalar.dma_start(
            out=wc_t[:], in_=wcT.rearrange("(g p) e -> p g e", p=128))
        # Defer wc's bytes until wg (a first-matmul gate) has fully landed.
        add_dep_helper(i_wc.ins, i_wg.ins, reason="defer wc behind wg")

        # Warm the PE's HAM clock gate (~3.4us of activity releases the
        # 1.2->2.4 GHz throttle) with dummy matmuls on a zeroed tile while
        # the startup DMAs are in flight.
        warm = wpool.tile([128, 512], F16)
        nc.vector.memset(warm[:], 0.0)
        warm_ps = pspool.tile([128, 512], F32, tag="warm", bufs=1)
        for _ in range(16):
            nc.tensor.matmul(warm_ps[:], warm[:, 0:128], warm[:, 0:512],
                             start=True, stop=True)

        h_prev = None
        for ci in range(NCH):
            t0 = ci * TC
            x_t = xpool.tile([128, NK, TC], F16, tag="x")
            i_x = nc.sync.dma_start(
                out=x_t[:],
                in_=xT[:, t0:t0 + TC].rearrange("(g p) t -> p g t", p=128),
            )
            # Keep the x1/x2 prefetch bytes out of the SDMA round-robin
            # until the two transfers gating the first matmul are done.
            if ci == 1:
                add_dep_helper(i_x.ins, i_wg.ins, reason="defer x1 behind wg")
            elif ci == 2:
                add_dep_helper(i_x.ins, i_wc.ins, reason="defer x2 behind wc")

            g_t = gcpool.tile([128, NE, TC], F32, tag="g")
            c_t = gcpool.tile([128, NE, TC], F32, tag="c")
            for w_t, b_t, out_t, func in (
                (wg_t, bg_t, g_t, mybir.ActivationFunctionType.Sigmoid),
                (wc_t, bc_t, c_t, mybir.ActivationFunctionType.Tanh),
            ):
                for eg in range(NE):
                    ps = pspool.tile([128, TC], F32, tag="ps")
                    for kg in range(NK):
                        nc.tensor.matmul(
                            ps[:],
                            w_t[:, kg, eg * 128:(eg + 1) * 128],
                            x_t[:, kg, :],
                            start=(kg == 0),
                            stop=(kg == NK - 1),
                        )
                    nc.scalar.activation(
                        out_t[:, eg, :], ps[:], func, bias=b_t[:, eg:eg + 1]
                    )

            h_t = hpool.tile([128, NE, TC], F32, tag="h")
            last = ci == NCH - 1
            for eg in range(NE):
                init = 0.0 if ci == 0 else h_prev[:, eg, TC - 1:TC]
                nc.vector.tensor_tensor_scan(
                    h_t[:, eg, :],
                    g_t[:, eg, :],
                    c_t[:, eg, :],
                    initial=init,
                    op0=mybir.AluOpType.mult,
                    op1=mybir.AluOpType.add,
                )
                if last:
                    # Per-group stores so the final store (the kernel-tail
                    # gate) only waits on the last scan and is small.
                    nc.sync.dma_start(
                        out=hT[eg * 128:(eg + 1) * 128, t0:t0 + TC],
                        in_=h_t[:, eg, :],
                    )
            if not last:
                nc.sync.dma_start(
                    out=hT[:, t0:t0 + TC].rearrange("(g p) t -> p g t", p=128),
                    in_=h_t[:],
                )
            h_prev = h_t

    nc.compile()
    return nc


def _get_compiled():
    global _compiled
    if _compiled is None:
        _compiled = _build()
    return _compiled


def make_in_maps(x, Wg, bg, Wc, bc):
    x = np.asarray(x, dtype=np.float32)
    xT = [np.ascontiguousarray(x[b].T.astype(np.float16)) for b in range(B)]
    in_maps = []
    for c in range(N_CORES):
        b, eh = divmod(c, 2)
        sl = slice(eh * E, (eh + 1) * E)
        in_maps.append({
            "xT": xT[b],
            "wgT": np.ascontiguousarray(
                np.asarray(Wg, np.float32)[sl].T.astype(np.float16)),
            "wcT": np.ascontiguousarray(
                np.asarray(Wc, np.float32)[sl].T.astype(np.float16)),
            "bias": np.ascontiguousarray(np.stack(
                [np.asarray(bg, np.float32)[sl].reshape(NE, 128),
                 np.asarray(bc, np.float32)[sl].reshape(NE, 128)],
            ).reshape(2 * NE, 128).T),
        })
    return in_maps


def assemble_output(results):
    out = np.empty((B, L, D), np.float32)
    for c in range(N_CORES):
        b, eh = divmod(c, 2)
        out[b, :, eh * E:(eh + 1) * E] = results[c]["hT"].T
    return out


def kernel(x, Wg, bg, Wc, bc, _trace=False, _trace_kwargs=None):
    nc = _get_compiled()
    in_maps = make_in_maps(x, Wg, bg, Wc, bc)
    res = run_bass_kernel_spmd(
        nc, in_maps, list(range(N_CORES)), trace=_trace,
        **(_trace_kwargs or {}),
    )
    out = assemble_output(res.results)
    if _trace:
        kernel.last_results = res
    return out
